# revision 2
# baseline (speedup 1.0000x reference)
# Malvar demosaic on 8 Trainium2 NeuronCores — pure data parallel (1 batch
# image per core).
#
# The wall-clock of kernel() is dominated by the axon tunnel (~40 MiB/s up,
# ~30 MiB/s down), so the design minimizes wire bytes and per-call jax work:
#   * input is quantized host-side to uint8 (32 MiB up instead of 128),
#   * the device computes only the 8 interpolated (conv) quarter-res planes
#     and returns them as uint8 (64 MiB down instead of 384); the 4
#     passthrough planes are pasted host-side from the original f32 input
#     during the download (they are verbatim input copies),
#   * the bass kernel is wrapped in bass_jit + bass_shard_map whose jax.jit
#     executable is cached across calls (no per-call re-trace/re-compile),
#     band matrices live on-device permanently, and no zero output buffers
#     are shipped.
# Device side: polyphase decomposition of each conv plane into banded
# [128x126] f32 matmuls on the TensorEngine (vertical taps across
# partitions, horizontal taps as strided rhs column reads), reflection
# padding folded into the first/last band matrices; ACT dequantizes u8
# input tiles (exact k/255), DVE re-quantizes PSUM with a single
# saturating round-to-nearest f32->u8 instruction (clip to [0,1] is free).
import hashlib
import queue as _queue
import threading
import numpy as np

# ---------------------------------------------------------------------------
# Problem constants (hardcoded per harness contract)
B, H, W = 8, 2048, 2048
N_CORES = 8
N_PHASE = H // 2        # 1024 phase rows
WN = W // 2             # 1024 phase cols
NCH = 512               # matmul moving free dim (one PSUM bank fp32)
NCHUNKS = WN // NCH


def MALVAR_KERNELS():
    g = np.array([[0, 0, -1, 0, 0], [0, 0, 2, 0, 0], [-1, 2, 4, 2, -1],
                  [0, 0, 2, 0, 0], [0, 0, -1, 0, 0]], np.float32) / 8.0
    col = np.array([[0, 0, 0.5, 0, 0], [0, -1, 0, -1, 0], [-1, 4, 5, 4, -1],
                    [0, -1, 0, -1, 0], [0, 0, 0.5, 0, 0]], np.float32) / 8.0
    row = np.array([[0, 0, -1, 0, 0], [0, -1, 4, -1, 0], [0.5, 0, 5, 0, 0.5],
                    [0, -1, 4, -1, 0], [0, 0, -1, 0, 0]], np.float32) / 8.0
    br = np.array([[0, 0, -1.5, 0, 0], [0, 2, 0, 2, 0], [-1.5, 0, 6, 0, -1.5],
                   [0, 2, 0, 2, 0], [0, 0, -1.5, 0, 0]], np.float32) / 8.0
    return {"g": g, "col": col, "row": row, "br": br}


# conv planes, in device output order: (out channel, row parity, col parity,
# kernel name)
CONV_OUTPUTS = [
    (1, 0, 0, "g"),    # green at R
    (2, 0, 0, "br"),   # blue  at R
    (0, 0, 1, "col"),  # red   at Gr
    (2, 0, 1, "row"),  # blue  at Gr
    (0, 1, 0, "row"),  # red   at Gb
    (2, 1, 0, "col"),  # blue  at Gb
    (0, 1, 1, "br"),   # red   at B
    (1, 1, 1, "g"),    # green at B
]
# passthrough planes (host-side): out[ch, 2i+di0, 2j+dj0] = x[2i+di0, 2j+dj0]
PASSTHROUGH_OUTPUTS = [(0, 0, 0), (1, 0, 1), (1, 1, 0), (2, 1, 1)]


def gen_passes(kernels=None):
    """Polyphase decomposition of the 8 conv planes.

    Returns a list of 8 dicts {ch, di0, dj0, passes}; passes is a list of
    {pr, pc, dcol, taps: {drow: coeff}}. Output plane value:
      out[i, j] = sum over passes, taps:
          coeff * phase[pr,pc][i + drow, j + dcol]
    for output full-res site (2i + di0, 2j + dj0).
    """
    if kernels is None:
        kernels = MALVAR_KERNELS()
    qs = []
    for ch, di0, dj0, kname in CONV_OUTPUTS:
        k = kernels[kname]
        groups = {}
        for u in range(-2, 3):
            for v in range(-2, 3):
                c = float(k[u + 2, v + 2])
                if c == 0.0:
                    continue
                pr = (di0 + u) % 2
                drow = (di0 + u - pr) // 2
                pc = (dj0 + v) % 2
                dcol = (dj0 + v - pc) // 2
                key = (pr, pc, dcol)
                groups.setdefault(key, {})
                groups[key][drow] = groups[key].get(drow, 0.0) + c
        passes = [{"pr": pr, "pc": pc, "dcol": dcol, "taps": taps}
                  for (pr, pc, dcol), taps in sorted(groups.items())]
        qs.append({"ch": ch, "di0": di0, "dj0": dj0, "passes": passes})
    return qs


def block_plan(n):
    """Row-block plan over n phase rows. Returns [(base, out0, M, cls)].

    Block covers output phase rows [out0, out0+M); its input tiles hold
    phase rows [base, base+128). cls: 0 first (reflect top), 1 interior,
    2 last (reflect bottom).
    """
    assert n >= 128
    plan = []
    out0 = 0
    while out0 < n:
        if out0 == 0:
            base, cls, M = 0, 0, 126
        elif out0 <= n - 127:
            base, cls, M = out0 - 1, 1, 126
        else:
            base, cls, M = n - 128, 2, n - out0
        plan.append((base, out0, M, cls))
        out0 += M
    return plan


def _class_geometry(n, cls):
    plan = block_plan(n)
    if cls == 0:
        return plan[0]
    if cls == 2:
        return plan[-1]
    interior = [b for b in plan if b[3] == 1]
    return interior[0] if interior else None


def gen_bands(n, cls, kernels=None):
    """Band (lhsT) matrices [128, 126] for every (q, pass) for block class
    cls. lhsT[k, m] = coeff so that psum[m, :] += sum_k lhsT[k, m]*tile[k, :]
    computes output phase row out0+m from tile rows (phase rows base+k),
    with reflection rows folded in."""
    qs = gen_passes(kernels)
    geo = _class_geometry(n, cls)
    bands = {}
    for qi, q in enumerate(qs):
        for pi, p in enumerate(q["passes"]):
            Bm = np.zeros((128, 126), np.float32)
            if geo is not None:
                base, out0, M, _ = geo
                pr = p["pr"]
                for m in range(126):
                    if out0 + m >= n:
                        continue
                    for drow, coeff in p["taps"].items():
                        r = out0 + m + drow
                        if r < 0:
                            r = -r - pr          # reflect top (same parity)
                        elif r >= n:
                            r = 2 * n - 1 - r - pr  # reflect bottom
                        k = r - base
                        assert 0 <= k < 128, (cls, qi, pi, m, drow, k)
                        Bm[k, m] += coeff
            bands[(qi, pi)] = Bm
    return bands


def n_conv_passes(kernels=None):
    return sum(len(q["passes"]) for q in gen_passes(kernels))


def build_bands_np(n, kernels=None):
    """[3, 128, G*126] f32 band tensor (G = total conv passes)."""
    qs = gen_passes(kernels)
    G = sum(len(q["passes"]) for q in qs)
    arr = np.zeros((3, 128, G * 126), np.float32)
    for cls in range(3):
        bands = gen_bands(n, cls, kernels)
        g = 0
        for qi, q in enumerate(qs):
            for pi in range(len(q["passes"])):
                arr[cls, :, g * 126:(g + 1) * 126] = bands[(qi, pi)]
                g += 1
    return np.ascontiguousarray(arr)


# ---------------------------------------------------------------------------
# Bass kernel (per core: x [H, W] u8  ->  y [8, N_PHASE, WN] u8)
def _make_bass_fn():
    from contextlib import ExitStack
    import concourse.tile as tile
    import concourse.mybir as mybir
    from concourse.bass2jax import bass_jit

    F32 = mybir.dt.float32
    U8 = mybir.dt.uint8
    INV255 = float(np.float32(1.0 / 255.0))

    qs = gen_passes()          # pass STRUCTURE is fixed (coeff values live
    gpi_of = {}                # in the bands input, structure in the code)
    g = 0
    for qi, q in enumerate(qs):
        for pi in range(len(q["passes"])):
            gpi_of[(qi, pi)] = g
            g += 1
    G = g
    plan = block_plan(N_PHASE)

    @bass_jit(trn_type="TRN2", enable_asserts=False, num_devices=N_CORES,
              disable_frame_to_traceback=True)
    def demosaic(nc, x, bands):
        y = nc.dram_tensor("y", [8, N_PHASE, WN], U8, kind="ExternalOutput")
        with ExitStack() as ctx:
            tc = ctx.enter_context(tile.TileContext(nc))
            in_pool = ctx.enter_context(tc.tile_pool(name="inp", bufs=2))
            band_pool = ctx.enter_context(tc.tile_pool(name="band", bufs=2))
            out_pool = ctx.enter_context(tc.tile_pool(name="outp", bufs=2))
            psum_pool = ctx.enter_context(tc.tile_pool(name="ps", bufs=8,
                                                       space="PSUM"))
            band_tiles = {}

            def get_band_tile(cls):
                if cls not in band_tiles:
                    bt = band_pool.tile([128, G * 126], F32, tag="bands")
                    nc.sync.dma_start(bt[:, :], bands[cls])
                    band_tiles[cls] = bt
                return band_tiles[cls]

            for (base, out0, M, cls) in plan:
                bt = get_band_tile(cls)
                tin = {}
                for pr in (0, 1):
                    su = in_pool.tile([128, W], U8, tag=f"s{pr}")
                    nc.sync.dma_start(
                        su[:, :], x[2 * base + pr: 2 * base + pr + 255: 2, :])
                    t = in_pool.tile([128, W + 4], F32, tag=f"t{pr}")
                    # exact dequant k/255 on ACT; cols shifted by +2 for pad
                    nc.scalar.activation(t[:, 2:W + 2], su[:, :],
                                         mybir.ActivationFunctionType.Copy,
                                         bias=0.0, scale=INV255)
                    # reflect-pad columns: tile col c <-> image col c-2
                    nc.scalar.copy(t[:, 0:1], t[:, 4:5])
                    nc.scalar.copy(t[:, 1:2], t[:, 3:4])
                    nc.scalar.copy(t[:, W + 2:W + 3], t[:, W:W + 1])
                    nc.scalar.copy(t[:, W + 3:W + 4], t[:, W - 1:W])
                    tin[pr] = t
                A = [out_pool.tile([128, WN], U8, tag=f"A{p}", name=f"A{p}")
                     for p in range(8)]
                for qi, q in enumerate(qs):
                    for c in range(NCHUNKS):
                        ps = psum_pool.tile([128, NCH], F32, tag="ps")
                        for pi, p in enumerate(q["passes"]):
                            gp = gpi_of[(qi, pi)]
                            lhsT = bt[:, gp * 126: gp * 126 + 126]
                            c0 = 2 * p["dcol"] + p["pc"] + 2 + 2 * NCH * c
                            rhs = tin[p["pr"]][:, c0: c0 + 2 * NCH - 1: 2]
                            nc.tensor.matmul(ps[0:126, :], lhsT, rhs,
                                             start=(pi == 0),
                                             stop=(pi == len(q["passes"]) - 1))
                        # saturating RNE f32->u8: u8 = clamp(round(255*v))
                        # (negative -> 0, >1 -> 255), so clip is implicit.
                        nc.vector.tensor_scalar(
                            A[qi][0:126, NCH * c: NCH * (c + 1)],
                            ps[0:126, :], 255.0, None, mybir.AluOpType.mult)
                for p in range(8):
                    nc.sync.dma_start(y[p, out0: out0 + M, :], A[p][0:M, :])
        return (y,)

    return demosaic


# ---------------------------------------------------------------------------
# Cached runner
_CACHE = {}


def _get_runner():
    if "runner" not in _CACHE:
        import jax
        from jax.sharding import Mesh, PartitionSpec as P, NamedSharding
        from concourse.bass2jax import bass_shard_map

        devs = jax.devices()[:N_CORES]
        mesh = Mesh(np.asarray(devs), ("core",))
        fn = _make_bass_fn()
        runner = bass_shard_map(fn, mesh=mesh,
                                in_specs=(P("core"), P()),
                                out_specs=(P("core"),))
        _CACHE["runner"] = runner
        _CACHE["mesh"] = mesh
        _CACHE["x_sharding"] = NamedSharding(mesh, P("core"))
        _CACHE["b_sharding"] = NamedSharding(mesh, P())
        _CACHE["bands"] = {}
        _CACHE["lut"] = (np.arange(256) * np.float32(1.0 / 255.0)).astype(
            np.float32)
    return _CACHE


def _get_bands_device(kernels):
    import jax
    cache = _get_runner()
    if kernels is None:
        key = "default"
    else:
        h = hashlib.md5()
        for name in ("g", "col", "row", "br"):
            h.update(np.ascontiguousarray(kernels[name], np.float32).tobytes())
        key = h.hexdigest()
    bands = cache["bands"]
    if key not in bands:
        bands_np = build_bands_np(N_PHASE, kernels)
        bands[key] = jax.device_put(bands_np, cache["b_sharding"])
    return bands[key]


def _quantize_input(bayer):
    """(B,1,H,W) f32 -> (B*H, W) u8 with round-to-nearest, matching the
    device's u8 = round(255*v) / host dequant v = u8/255 convention."""
    x = bayer.reshape(B * H, W)
    q = x * np.float32(255.0)
    np.rint(q, out=q)
    np.clip(q, 0.0, 255.0, out=q)
    return q.astype(np.uint8)


def _paste_passthrough(out, bayer):
    for b in range(B):
        img = bayer[b, 0]
        for ch, di, dj in PASSTHROUGH_OUTPUTS:
            out[b, ch, di::2, dj::2] = np.clip(img[di::2, dj::2], 0.0, 1.0)


def _assemble_conv(out, b, planes_u8, lut):
    for p, (ch, di, dj, _k) in enumerate(CONV_OUTPUTS):
        out[b, ch, di::2, dj::2] = lut[planes_u8[p]]


def kernel(**inputs) -> np.ndarray:
    bayer = np.asarray(inputs["bayer"], dtype=np.float32)
    b, c1, h, w = bayer.shape
    assert (b, c1, h, w) == (B, 1, H, W), bayer.shape

    kernels = None
    if "k_g_at_rb" in inputs:
        kernels = {
            "g": np.asarray(inputs["k_g_at_rb"], np.float32).reshape(5, 5),
            "col": np.asarray(inputs["k_rb_at_g_col"], np.float32).reshape(5, 5),
            "row": np.asarray(inputs["k_rb_at_g_row"], np.float32).reshape(5, 5),
            "br": np.asarray(inputs["k_rb_at_br"], np.float32).reshape(5, 5),
        }

    import jax
    cache = _get_runner()
    bands_d = _get_bands_device(kernels)

    xq = _quantize_input(bayer)
    xd = jax.device_put(xq, cache["x_sharding"])
    (yd,) = cache["runner"](xd, bands_d)

    out = np.empty((B, 3, H, W), np.float32)

    # stream the download: fetch shards in a worker thread (the tunnel is
    # the bottleneck and serializes anyway), assemble in the main thread.
    qq = _queue.Queue()

    def _fetch():
        try:
            shards = sorted(yd.addressable_shards,
                            key=lambda s: (s.index[0].start or 0))
            for s in shards:
                img = (s.index[0].start or 0) // 8
                qq.put((img, np.asarray(s.data)))
            qq.put(None)
        except BaseException as e:  # surface errors in the main thread
            qq.put(e)

    th = threading.Thread(target=_fetch, daemon=True)
    th.start()

    _paste_passthrough(out, bayer)  # overlapped with the download

    lut = cache["lut"]
    while True:
        item = qq.get()
        if item is None:
            break
        if isinstance(item, BaseException):
            raise item
        img, planes = item
        _assemble_conv(out, img, planes, lut)
    th.join()
    return out


if __name__ == "__main__":
    qs = gen_passes()
    for q in qs:
        print(q["ch"], q["di0"], q["dj0"], "passes:", len(q["passes"]))
    print("total conv passes:", n_conv_passes())
    print("plan n=1024:", block_plan(1024))


# revision 7
# speedup vs baseline: 1.6753x; 1.6753x over previous
# Malvar demosaic on 8 Trainium2 NeuronCores — pure data parallel (1 batch
# image per core).
#
# The wall-clock of kernel() is dominated by the axon tunnel (~40 MiB/s up,
# ~30 MiB/s down), so the design minimizes wire bytes and per-call jax work:
#   * input is quantized host-side to uint8 (32 MiB up instead of 128),
#   * the device computes only the 8 interpolated (conv) quarter-res planes
#     and returns them as uint8 (64 MiB down instead of 384); the 4
#     passthrough planes are pasted host-side from the original f32 input
#     during the download (they are verbatim input copies),
#   * the bass kernel is wrapped in bass_jit + bass_shard_map whose jax.jit
#     executable is cached across calls (no per-call re-trace/re-compile),
#     band matrices live on-device permanently, and no zero output buffers
#     are shipped.
# Device side: polyphase decomposition of each conv plane into banded
# [128x126] f32 matmuls on the TensorEngine (vertical taps across
# partitions, horizontal taps as strided rhs column reads), reflection
# padding folded into the first/last band matrices; ACT dequantizes u8
# input tiles (exact k/255), DVE re-quantizes PSUM with a single
# saturating round-to-nearest f32->u8 instruction (clip to [0,1] is free).
import hashlib
import numpy as np

# ---------------------------------------------------------------------------
# Problem constants (hardcoded per harness contract)
B, H, W = 8, 2048, 2048
N_CORES = 8
N_PHASE = H // 2        # 1024 phase rows
WN = W // 2             # 1024 phase cols
NCH = 512               # matmul moving free dim (one PSUM bank fp32)
NCHUNKS = WN // NCH


def MALVAR_KERNELS():
    g = np.array([[0, 0, -1, 0, 0], [0, 0, 2, 0, 0], [-1, 2, 4, 2, -1],
                  [0, 0, 2, 0, 0], [0, 0, -1, 0, 0]], np.float32) / 8.0
    col = np.array([[0, 0, 0.5, 0, 0], [0, -1, 0, -1, 0], [-1, 4, 5, 4, -1],
                    [0, -1, 0, -1, 0], [0, 0, 0.5, 0, 0]], np.float32) / 8.0
    row = np.array([[0, 0, -1, 0, 0], [0, -1, 4, -1, 0], [0.5, 0, 5, 0, 0.5],
                    [0, -1, 4, -1, 0], [0, 0, -1, 0, 0]], np.float32) / 8.0
    br = np.array([[0, 0, -1.5, 0, 0], [0, 2, 0, 2, 0], [-1.5, 0, 6, 0, -1.5],
                   [0, 2, 0, 2, 0], [0, 0, -1.5, 0, 0]], np.float32) / 8.0
    return {"g": g, "col": col, "row": row, "br": br}


# conv planes, in device output order: (out channel, row parity, col parity,
# kernel name)
CONV_OUTPUTS = [
    (1, 0, 0, "g"),    # green at R
    (2, 0, 0, "br"),   # blue  at R
    (0, 0, 1, "col"),  # red   at Gr
    (2, 0, 1, "row"),  # blue  at Gr
    (0, 1, 0, "row"),  # red   at Gb
    (2, 1, 0, "col"),  # blue  at Gb
    (0, 1, 1, "br"),   # red   at B
    (1, 1, 1, "g"),    # green at B
]
# passthrough planes (host-side): out[ch, 2i+di0, 2j+dj0] = x[2i+di0, 2j+dj0]
PASSTHROUGH_OUTPUTS = [(0, 0, 0), (1, 0, 1), (1, 1, 0), (2, 1, 1)]


def gen_passes(kernels=None):
    """Polyphase decomposition of the 8 conv planes.

    Returns a list of 8 dicts {ch, di0, dj0, passes}; passes is a list of
    {pr, pc, dcol, taps: {drow: coeff}}. Output plane value:
      out[i, j] = sum over passes, taps:
          coeff * phase[pr,pc][i + drow, j + dcol]
    for output full-res site (2i + di0, 2j + dj0).
    """
    if kernels is None:
        kernels = MALVAR_KERNELS()
    qs = []
    for ch, di0, dj0, kname in CONV_OUTPUTS:
        k = kernels[kname]
        groups = {}
        for u in range(-2, 3):
            for v in range(-2, 3):
                c = float(k[u + 2, v + 2])
                if c == 0.0:
                    continue
                pr = (di0 + u) % 2
                drow = (di0 + u - pr) // 2
                pc = (dj0 + v) % 2
                dcol = (dj0 + v - pc) // 2
                key = (pr, pc, dcol)
                groups.setdefault(key, {})
                groups[key][drow] = groups[key].get(drow, 0.0) + c
        passes = [{"pr": pr, "pc": pc, "dcol": dcol, "taps": taps}
                  for (pr, pc, dcol), taps in sorted(groups.items())]
        qs.append({"ch": ch, "di0": di0, "dj0": dj0, "passes": passes})
    return qs


def block_plan(n):
    """Row-block plan over n phase rows. Returns [(base, out0, M, cls)].

    Block covers output phase rows [out0, out0+M); its input tiles hold
    phase rows [base, base+128). cls: 0 first (reflect top), 1 interior,
    2 last (reflect bottom).
    """
    assert n >= 128
    plan = []
    out0 = 0
    while out0 < n:
        if out0 == 0:
            base, cls, M = 0, 0, 126
        elif out0 <= n - 127:
            base, cls, M = out0 - 1, 1, 126
        else:
            base, cls, M = n - 128, 2, n - out0
        plan.append((base, out0, M, cls))
        out0 += M
    return plan


def _class_geometry(n, cls):
    plan = block_plan(n)
    if cls == 0:
        return plan[0]
    if cls == 2:
        return plan[-1]
    interior = [b for b in plan if b[3] == 1]
    return interior[0] if interior else None


def gen_bands(n, cls, kernels=None):
    """Band (lhsT) matrices [128, 126] for every (q, pass) for block class
    cls. lhsT[k, m] = coeff so that psum[m, :] += sum_k lhsT[k, m]*tile[k, :]
    computes output phase row out0+m from tile rows (phase rows base+k),
    with reflection rows folded in."""
    qs = gen_passes(kernels)
    geo = _class_geometry(n, cls)
    bands = {}
    for qi, q in enumerate(qs):
        for pi, p in enumerate(q["passes"]):
            Bm = np.zeros((128, 126), np.float32)
            if geo is not None:
                base, out0, M, _ = geo
                pr = p["pr"]
                for m in range(126):
                    if out0 + m >= n:
                        continue
                    for drow, coeff in p["taps"].items():
                        r = out0 + m + drow
                        if r < 0:
                            r = -r - pr          # reflect top (same parity)
                        elif r >= n:
                            r = 2 * n - 1 - r - pr  # reflect bottom
                        k = r - base
                        assert 0 <= k < 128, (cls, qi, pi, m, drow, k)
                        Bm[k, m] += coeff
            bands[(qi, pi)] = Bm
    return bands


def n_conv_passes(kernels=None):
    return sum(len(q["passes"]) for q in gen_passes(kernels))


def build_bands_np(n, kernels=None):
    """[3, 128, G*126] f32 band tensor (G = total conv passes)."""
    qs = gen_passes(kernels)
    G = sum(len(q["passes"]) for q in qs)
    arr = np.zeros((3, 128, G * 126), np.float32)
    for cls in range(3):
        bands = gen_bands(n, cls, kernels)
        g = 0
        for qi, q in enumerate(qs):
            for pi in range(len(q["passes"])):
                arr[cls, :, g * 126:(g + 1) * 126] = bands[(qi, pi)]
                g += 1
    return np.ascontiguousarray(arr)


# ---------------------------------------------------------------------------
# Bass kernel (per core: x [H, W] u8  ->  y [8, N_PHASE, WN] u8)
def _make_bass_fn():
    from contextlib import ExitStack
    import concourse.tile as tile
    import concourse.mybir as mybir
    from concourse.bass2jax import bass_jit

    F32 = mybir.dt.float32
    U8 = mybir.dt.uint8
    INV255 = float(np.float32(1.0 / 255.0))

    qs = gen_passes()          # pass STRUCTURE is fixed (coeff values live
    gpi_of = {}                # in the bands input, structure in the code)
    g = 0
    for qi, q in enumerate(qs):
        for pi in range(len(q["passes"])):
            gpi_of[(qi, pi)] = g
            g += 1
    G = g
    plan = block_plan(N_PHASE)

    @bass_jit(trn_type="TRN2", enable_asserts=False, num_devices=N_CORES,
              disable_frame_to_traceback=True)
    def demosaic(nc, x, bands):
        y = nc.dram_tensor("y", [8, N_PHASE, WN], U8, kind="ExternalOutput")
        with ExitStack() as ctx:
            tc = ctx.enter_context(tile.TileContext(nc))
            in_pool = ctx.enter_context(tc.tile_pool(name="inp", bufs=2))
            band_pool = ctx.enter_context(tc.tile_pool(name="band", bufs=2))
            out_pool = ctx.enter_context(tc.tile_pool(name="outp", bufs=2))
            psum_pool = ctx.enter_context(tc.tile_pool(name="ps", bufs=8,
                                                       space="PSUM"))
            band_tiles = {}

            def get_band_tile(cls):
                if cls not in band_tiles:
                    bt = band_pool.tile([128, G * 126], F32, tag="bands")
                    nc.sync.dma_start(bt[:, :], bands[cls])
                    band_tiles[cls] = bt
                return band_tiles[cls]

            for (base, out0, M, cls) in plan:
                bt = get_band_tile(cls)
                tin = {}
                for pr in (0, 1):
                    su = in_pool.tile([128, W], U8, tag=f"s{pr}")
                    nc.sync.dma_start(
                        su[:, :], x[2 * base + pr: 2 * base + pr + 255: 2, :])
                    t = in_pool.tile([128, W + 4], F32, tag=f"t{pr}")
                    # exact dequant k/255 on ACT; cols shifted by +2 for pad
                    nc.scalar.activation(t[:, 2:W + 2], su[:, :],
                                         mybir.ActivationFunctionType.Copy,
                                         bias=0.0, scale=INV255)
                    # reflect-pad columns: tile col c <-> image col c-2
                    nc.scalar.copy(t[:, 0:1], t[:, 4:5])
                    nc.scalar.copy(t[:, 1:2], t[:, 3:4])
                    nc.scalar.copy(t[:, W + 2:W + 3], t[:, W:W + 1])
                    nc.scalar.copy(t[:, W + 3:W + 4], t[:, W - 1:W])
                    tin[pr] = t
                A = [out_pool.tile([128, WN], U8, tag=f"A{p}", name=f"A{p}")
                     for p in range(8)]
                for qi, q in enumerate(qs):
                    for c in range(NCHUNKS):
                        ps = psum_pool.tile([128, NCH], F32, tag="ps")
                        for pi, p in enumerate(q["passes"]):
                            gp = gpi_of[(qi, pi)]
                            lhsT = bt[:, gp * 126: gp * 126 + 126]
                            c0 = 2 * p["dcol"] + p["pc"] + 2 + 2 * NCH * c
                            rhs = tin[p["pr"]][:, c0: c0 + 2 * NCH - 1: 2]
                            nc.tensor.matmul(ps[0:126, :], lhsT, rhs,
                                             start=(pi == 0),
                                             stop=(pi == len(q["passes"]) - 1))
                        # saturating RNE f32->u8: u8 = clamp(round(255*v))
                        # (negative -> 0, >1 -> 255), so clip is implicit.
                        nc.vector.tensor_scalar(
                            A[qi][0:126, NCH * c: NCH * (c + 1)],
                            ps[0:126, :], 255.0, None, mybir.AluOpType.mult)
                for p in range(8):
                    nc.sync.dma_start(y[p, out0: out0 + M, :], A[p][0:M, :])
        return (y,)

    return demosaic


# ---------------------------------------------------------------------------
# Cached runner
_CACHE = {}


def _get_runner():
    if "runner" not in _CACHE:
        import jax
        from jax.sharding import Mesh, PartitionSpec as P, NamedSharding
        from concourse.bass2jax import bass_shard_map

        devs = jax.devices()[:N_CORES]
        mesh = Mesh(np.asarray(devs), ("core",))
        fn = _make_bass_fn()
        runner = bass_shard_map(fn, mesh=mesh,
                                in_specs=(P("core"), P()),
                                out_specs=(P("core"),))
        _CACHE["runner"] = runner
        _CACHE["mesh"] = mesh
        _CACHE["x_sharding"] = NamedSharding(mesh, P("core"))
        _CACHE["b_sharding"] = NamedSharding(mesh, P())
        _CACHE["bands"] = {}
        _CACHE["lut"] = (np.arange(256) * np.float32(1.0 / 255.0)).astype(
            np.float32)
    return _CACHE


def _get_bands_device(kernels):
    import jax
    cache = _get_runner()
    if kernels is None:
        key = "default"
    else:
        h = hashlib.md5()
        for name in ("g", "col", "row", "br"):
            h.update(np.ascontiguousarray(kernels[name], np.float32).tobytes())
        key = h.hexdigest()
    bands = cache["bands"]
    if key not in bands:
        bands_np = build_bands_np(N_PHASE, kernels)
        bands[key] = jax.device_put(bands_np, cache["b_sharding"])
    return bands[key]


def _quantize_input(bayer):
    """(B,1,H,W) f32 -> (B*H, W) u8 with round-to-nearest, matching the
    device's u8 = round(255*v) / host dequant v = u8/255 convention."""
    x = bayer.reshape(B * H, W)
    q = x * np.float32(255.0)
    np.rint(q, out=q)
    np.clip(q, 0.0, 255.0, out=q)
    return q.astype(np.uint8)


def _paste_passthrough(out, bayer):
    for b in range(B):
        img = bayer[b, 0]
        for ch, di, dj in PASSTHROUGH_OUTPUTS:
            out[b, ch, di::2, dj::2] = np.clip(img[di::2, dj::2], 0.0, 1.0)


def _assemble_conv(out, b, planes_u8, lut):
    for p, (ch, di, dj, _k) in enumerate(CONV_OUTPUTS):
        out[b, ch, di::2, dj::2] = lut[planes_u8[p]]


def kernel(**inputs) -> np.ndarray:
    import os
    import time
    dbg = os.environ.get("DEMOSAIC_DEBUG", "0") == "1"
    tlog = []
    t00 = time.time()

    def mark(name):
        if dbg:
            tlog.append((name, time.time() - t00))

    bayer = np.asarray(inputs["bayer"], dtype=np.float32)
    b, c1, h, w = bayer.shape
    assert (b, c1, h, w) == (B, 1, H, W), bayer.shape

    kernels = None
    if "k_g_at_rb" in inputs:
        kernels = {
            "g": np.asarray(inputs["k_g_at_rb"], np.float32).reshape(5, 5),
            "col": np.asarray(inputs["k_rb_at_g_col"], np.float32).reshape(5, 5),
            "row": np.asarray(inputs["k_rb_at_g_row"], np.float32).reshape(5, 5),
            "br": np.asarray(inputs["k_rb_at_br"], np.float32).reshape(5, 5),
        }

    import jax
    cache = _get_runner()
    bands_d = _get_bands_device(kernels)
    mark("setup")

    xq = _quantize_input(bayer)
    mark("quantize")
    xd = jax.device_put(xq, cache["x_sharding"])
    xd.block_until_ready() if dbg else None
    mark("h2d")
    (yd,) = cache["runner"](xd, bands_d)
    mark("dispatch")

    out = np.empty((B, 3, H, W), np.float32)

    # start the D2H transfer in jax's C++ runtime (no GIL contention with
    # the numpy work below); np.asarray then hits the prefetched copy.
    try:
        yd.copy_to_host_async()
    except Exception:
        pass
    _paste_passthrough(out, bayer)  # overlapped with the download
    mark("passthrough")

    yq = np.asarray(yd)  # (B*8, N_PHASE, WN) u8
    mark("download")
    lut = cache["lut"]
    for img in range(B):
        _assemble_conv(out, img, yq[8 * img: 8 * img + 8], lut)
    mark("assemble")
    if dbg:
        prev = 0.0
        for name, t in tlog:
            print(f"  [kernel] {name}: +{t - prev:.2f}s (cum {t:.2f}s)")
            prev = t
    return out


if __name__ == "__main__":
    qs = gen_passes()
    for q in qs:
        print(q["ch"], q["di0"], q["dj0"], "passes:", len(q["passes"]))
    print("total conv passes:", n_conv_passes())
    print("plan n=1024:", block_plan(1024))
